# revision 33
# baseline (speedup 1.0000x reference)
"""Trainium2 Bass kernel for causal cosine-sim attention block (8 cores), v6.

Reference computation:
  x [2, 2048, 1024] fp32
  xn = LayerNorm(x) * ln_w + ln_b
  qkv = xn @ W_qkv -> q, k, v   (16 heads x 64)
  q, k l2-normalized per head-dim; sim = (q.k) * 8, causal mask, softmax
  o = attn @ v ; out = o @ W_out   [2, 2048, 1024] fp32

Sharding (8 cores):
  - QKV projection + attention: head-parallel. Core c owns heads {2c, 2c+1}
    (column-sharded W_qkv).
  - Out projection: token-parallel over strided 128-token granules:
    granule g (tokens [128g, 128g+128)) of each batch-half belongs to core
    g % 8. Four small AllToAlls (one per batch-half) exchange head-sharded
    o^T for token granules, each overlapping subsequent attention compute.

v6 structure notes:
  - LN mean-subtraction is folded into W on the host (column-centered W);
    no on-chip rank-1 correction at all. A 385th W column of 1/1024 makes
    the QKV psum's last column the per-token mean (needed for variance).
  - rstd (for the V scale; it cancels for q,k under l2-norm) comes from
    sum(x^2) computed on DVE from x rows; var = ssq/1024 - mu^2.
  - x is fed twice from HBM: token-row tiles (for x^2) and transposed
    stripes x^T (matmul lhsT) -- no on-chip x transposes.
  - q is scaled by 1/||q|| on DVE; k stays raw and 8/||k|| rides the Exp
    activation's per-partition scale operand.
  - Softmax denominators (from the [V|1] PV matmul) are normalized via
    reciprocal + K=1 ones-matmul broadcast + DVE multiply.
  - PE warm-up: dummy matmuls at kernel start trip the HAM clock gate
    (cold 1.2 GHz -> warm 2.4 GHz) while input DMAs run.
  - Schedule: QKV(b0) -> [attention(b0) || QKV(b1)] -> [attention(b1) ||
    A2A(b0)+outproj(b0)] -> per-half A2A(b1)/outproj(b1) pipeline.
"""

import numpy as np

import concourse.bass as bass
import concourse.mybir as mybir
import concourse.tile as tile
from concourse import bacc
from concourse.bass import ts, ds

F32 = mybir.dt.float32
BF16 = mybir.dt.bfloat16
FP8 = mybir.dt.float8e4

NCORES = 8
DIM = 1024
HEADS = 16
DHEAD = 64
INNER = HEADS * DHEAD          # 1024
B = 2
N = 2048
NTOK = B * N                   # 4096
HLOC = HEADS // NCORES         # 2 heads per core
QKV_COLS = 3 * HLOC * DHEAD    # 384
EPS = 1e-5
SCALE = 8.0
P = 128
KT_PER_B = N // P              # 16 token tiles per batch
QB_PER_B = N // 512            # 4 q-blocks (512) per batch
GRAN = 128                     # out-proj token granule
AluOp = mybir.AluOpType
Act = mybir.ActivationFunctionType
AxX = mybir.AxisListType.X


def build_kernel():
    nc = bacc.Bacc("TRN2", target_bir_lowering=False, debug=False,
                   num_devices=NCORES)

    x_rows = nc.dram_tensor("x_rows", [NTOK, DIM], FP8,
                        kind="ExternalInput")
    x_tr = nc.dram_tensor("x_tr", [P, DIM // P, NTOK], BF16,
                          kind="ExternalInput")
    w_qkv = nc.dram_tensor("w_qkv", [P, DIM // P, QKV_COLS + 1], BF16,
                           kind="ExternalInput")
    w_out = nc.dram_tensor("w_out", [P, INNER // P, DIM], BF16,
                           kind="ExternalInput")
    y_out = nc.dram_tensor("y_out", [B, 2, GRAN, DIM], BF16,
                           kind="ExternalOutput")

    with tile.TileContext(nc) as tc:
        _body(nc, tc, x_rows, x_tr, w_qkv, w_out, y_out)
    nc.compile()
    return nc


def _body(nc, tc, x_rows, x_tr, w_qkv, w_out, y_out):
    import contextlib
    ctx = contextlib.ExitStack()
    with ctx:
        persist = ctx.enter_context(tc.tile_pool(name="persist", bufs=1))
        xt_pool = ctx.enter_context(tc.tile_pool(name="xt", bufs=2))
        xr_pool = ctx.enter_context(tc.tile_pool(name="xr", bufs=4))
        xr1_pool = ctx.enter_context(tc.tile_pool(name="xr1", bufs=2))
        sq_pool = ctx.enter_context(tc.tile_pool(name="sqd", bufs=2))
        qk_pool = ctx.enter_context(tc.tile_pool(name="qks", bufs=2))
        small = ctx.enter_context(tc.tile_pool(name="small", bufs=4))
        mm_ps = ctx.enter_context(
            tc.tile_pool(name="mmps", bufs=2, space="PSUM"))
        st_ps_pool = ctx.enter_context(
            tc.tile_pool(name="stps", bufs=2, space="PSUM"))
        o_ps_pool = ctx.enter_context(
            tc.tile_pool(name="ops", bufs=1, space="PSUM"))
        bc_ps_pool = ctx.enter_context(
            tc.tile_pool(name="bcps", bufs=1, space="PSUM"))
        warm_ps_pool = ctx.enter_context(
            tc.tile_pool(name="warmps", bufs=1, space="PSUM"))
        e_pool = ctx.enter_context(tc.tile_pool(name="epool", bufs=4))
        oU_pool = ctx.enter_context(tc.tile_pool(name="oU", bufs=2))
        oT_pool = ctx.enter_context(tc.tile_pool(name="oT", bufs=2))
        oA_pool = ctx.enter_context(tc.tile_pool(name="oA", bufs=2))
        out_pool = ctx.enter_context(tc.tile_pool(name="outp", bufs=2))
        norm_pool = ctx.enter_context(tc.tile_pool(name="norm", bufs=2))
        dram = ctx.enter_context(tc.tile_pool(name="dram", bufs=1,
                                              space="DRAM"))

        # ---------------- persistent SBUF ----------------
        w_qkv_sb = persist.tile([P, DIM // P, QKV_COLS + 1], BF16)
        w_out_sb = persist.tile([P, INNER // P, DIM], BF16)
        qkT = persist.tile([P, 2, B, N], BF16)                    # 16 KB
        v_sb = persist.tile([P, B, KT_PER_B, HLOC, DHEAD + 1], BF16)
        rk8_all = persist.tile([P, B, KT_PER_B, HLOC], F32)
        ones1 = persist.tile([DHEAD + 1, DHEAD], BF16)
        warm_a = persist.tile([P, P], BF16)
        warm_b = persist.tile([P, 512], BF16)
        tri = persist.tile([P, P], BF16)
        den = persist.tile([DHEAD + 1, 2, 512], F32)
        rden = persist.tile([DHEAD + 1, 2, 512], BF16)

        # upfront DMAs (Sync queue) + constants; w_out is deferred (only
        # needed ~2/3 into the kernel) so it doesn't block x loads.
        nc.scalar.dma_start(w_qkv_sb[:], w_qkv.ap())
        nc.vector.memset(ones1[:], 1.0)
        nc.vector.memset(warm_a[:], 0.0)
        nc.vector.memset(warm_b[:], 0.0)
        nc.vector.memset(den[:], 1.0)
        nc.vector.memset(v_sb[:, :, :, :, DHEAD], 1.0)
        nc.vector.memset(tri[:], 1.0)
        nc.gpsimd.affine_select(
            out=tri[:], in_=tri[:], pattern=[[1, P]],
            compare_op=AluOp.is_ge, fill=0.0, base=0,
            channel_multiplier=-1)

        # x^T in per-512-token-group tiles: QKV tile ti waits only on its
        # own group's 1MB DMA. Order: xt(b0), xr(b0) on Sync; xr(b1) on
        # Scalar; xt(b1) on Sync behind xr(b0).
        xr_view = x_rows.ap().rearrange("(n p) d -> p n d", p=P)
        xtg = [[persist.tile([P, DIM // P, 512], BF16, name=f"xtg{bi}{g}")
                for g in range(4)] for bi in range(B)]
        xr0 = [None] * 4
        xr1 = [None] * 4

        def xr_load(bi, g):
            pool = xr_pool if bi == 0 else xr1_pool
            xr = pool.tile([P, 4, DIM], FP8, tag=f"xr{bi}",
                           name=f"xr{bi}{g}")
            nc.scalar.dma_start(xr[:],
                                xr_view[:, ds(bi * KT_PER_B + 4 * g, 4), :])
            (xr0 if bi == 0 else xr1)[g] = xr

        for g in range(4):
            nc.sync.dma_start(xtg[0][g][:],
                              x_tr.ap()[:, :, ds(g * 512, 512)])
        for g in range(4):
            nc.sync.dma_start(xtg[1][g][:],
                              x_tr.ap()[:, :, ds(N + g * 512, 512)])
        xr_load(1, 0)
        xr_load(1, 1)
        for g in range(4):
            xr_load(0, g)

        # PE warm-up: dummy matmuls while DMAs land (~9 us of PE busy)
        warm_ps = warm_ps_pool.tile([P, 512], F32, tag="warm")
        for i in range(20):
            nc.tensor.matmul(warm_ps[:], lhsT=warm_a[:], rhs=warm_b[:],
                             start=True, stop=True)

        # DRAM staging for the 4 AllToAlls (one per batch-half)
        cc_in = [[None] * 2 for _ in range(B)]
        cc_out = [[None] * 2 for _ in range(B)]
        for bi in range(B):
            for h in range(2):
                cc_in[bi][h] = dram.tile([NCORES, P, GRAN], BF16,
                                         name=f"cci{bi}{h}")
                cc_out[bi][h] = dram.tile([NCORES, P, GRAN], BF16,
                                          name=f"cco{bi}{h}")

        oT_b = [None] * B      # per-batch o^T (normalized, bf16)
        st = {}                # per-batch QKV staging tiles
        ssqx_all = persist.tile([P, B, KT_PER_B], F32)

        # ---- x^2: batch 0 on GpSimd(square)+DVE(reduce), batch 1 on ACT
        # Square+accum. Splits ~40us of elementwise work across engines;
        # all of it runs before the first Exp (no ACT table thrash).
        def x2_square(bi, t):
            g, j = divmod(t, 4)
            if bi == 1 and t in (4, 8):
                xr_load(1, 2 + (t == 8))
            xr = (xr0 if bi == 0 else xr1)[g]
            dump = sq_pool.tile([P, DIM], BF16, tag="sqd")
            nc.scalar.activation(dump[:], xr[:, j, :], Act.Square,
                                 accum_out=ssqx_all[:, bi, t:t + 1])

        # ---------------- stage A: QKV for one 128-token tile ------------
        # Per tile: matmuls + raw evacs + squared sums only (no ACT, no
        # reciprocal) -- the norm math is batched per batch to avoid ACT
        # table thrash and per-tile reciprocal overhead.
        def qkv_tile(bi, ti):
            if ti == 0:
                st[bi] = dict(
                    qk_bf=qk_pool.tile([P, KT_PER_B, 2 * P], BF16,
                                       tag="qkbf", name=f"qkbf{bi}"),
                    mu=small.tile([P, KT_PER_B], F32, tag="muall",
                                  name=f"mu{bi}"),
                    ssq=small.tile([P, KT_PER_B, 4], F32, tag="ssqall",
                                   name=f"ssq{bi}"),
                )
            s = st[bi]

            qkv_ps = mm_ps.tile([P, QKV_COLS + 1], F32, tag="mm")
            for o in range(DIM // P):
                nc.tensor.matmul(qkv_ps[:],
                                 lhsT=xtg[bi][ti // 4][:, o, ts(ti % 4, P)],
                                 rhs=w_qkv_sb[:, o, :],
                                 start=(o == 0), stop=(o == DIM // P - 1))

            # raw evacs: mu col + q|k bf16 on DVE, v bf16 on ACT
            nc.vector.tensor_copy(s["mu"][:, ti:ti + 1],
                                  qkv_ps[:, QKV_COLS:QKV_COLS + 1])
            nc.vector.tensor_copy(s["qk_bf"][:, ti, :], qkv_ps[:, 0:2 * P])
            nc.vector.tensor_copy(
                v_sb[:, bi, ti, :, 0:DHEAD],
                qkv_ps[:, 2 * P:2 * P + 2 * DHEAD]
                .rearrange("p (h d) -> p h d", d=DHEAD))

            # squared norms per 64-col group
            sq = sq_pool.tile([P, 2 * P], BF16, tag="sq")
            nc.vector.tensor_tensor(sq[:], s["qk_bf"][:, ti, :],
                                    s["qk_bf"][:, ti, :], AluOp.mult)
            nc.vector.reduce_sum(
                s["ssq"][:, ti, :],
                sq[:].rearrange("p (j d) -> p j d", d=DHEAD), axis=AxX)

        # critical half of batch-end: q-norm scales + transposes
        def qkv_bend_qk(bi):
            s = st[bi]
            rq = small.tile([P, KT_PER_B, 2], F32, tag="rq")
            nc.scalar.activation(rq[:], s["ssq"][:, :, 0:2], Act.Sqrt)
            nc.vector.tensor_scalar_max(rq[:], rq[:], 1e-12)
            nc.vector.reciprocal_approx_fast(rq[:], rq[:])
            rk = small.tile([P, KT_PER_B, 2], F32, tag="rk")
            nc.scalar.activation(rk[:], s["ssq"][:, :, 2:4], Act.Sqrt,
                                 scale=1.0 / (SCALE * SCALE))
            nc.vector.tensor_scalar_max(rk[:], rk[:], 1e-12 / SCALE)
            nc.vector.reciprocal_approx_fast(rk8_all[:, bi, :, :], rk[:])
            for ti in range(KT_PER_B):
                for hh in range(HLOC):
                    nc.vector.tensor_scalar_mul(
                        s["qk_bf"][:, ti, ts(hh, DHEAD)],
                        s["qk_bf"][:, ti, ts(hh, DHEAD)],
                        rq[:, ti, hh:hh + 1])
                eng = nc.sync if ti % 2 == 0 else nc.scalar
                eng.dma_start_transpose(qkT[:, :, bi, ts(ti, P)],
                                        s["qk_bf"][:, ti, :])

        # relaxed half: rstd chain + v scales (needed by first PV only)
        def qkv_bend_v(bi):
            s = st[bi]
            musq = small.tile([P, KT_PER_B], F32, tag="musq")
            nc.vector.tensor_tensor(musq[:], s["mu"][:], s["mu"][:],
                                    AluOp.mult)
            varr = small.tile([P, KT_PER_B], F32, tag="varr")
            nc.vector.tensor_scalar(varr[:], ssqx_all[:, bi, :], 1.0 / DIM,
                                    EPS, AluOp.mult, AluOp.add)
            nc.vector.tensor_tensor(varr[:], varr[:], musq[:],
                                    AluOp.subtract)
            rstd = small.tile([P, KT_PER_B], F32, tag="rstd")
            nc.scalar.activation(rstd[:], varr[:], Act.Sqrt)
            nc.vector.reciprocal_approx_fast(rstd[:], rstd[:])
            for ti in range(KT_PER_B):
                nc.vector.tensor_scalar_mul(
                    v_sb[:, bi, ti, :, 0:DHEAD],
                    v_sb[:, bi, ti, :, 0:DHEAD], rstd[:, ti:ti + 1])

        # ---------------- stage B: attention for one 512-q block ---------
        def attn_qblock(bi, qb, oU):
            o_ps = []
            for hh in range(HLOC):
                o_ps.append(o_ps_pool.tile([1 + DHEAD, 512], F32,
                                           tag=f"ops{hh}", name=f"ops{hh}"))
            nkt = 4 * (qb + 1)
            for kt in range(nkt):
                d = kt - 4 * qb
                c0 = max(d, 0) * P
                for hh in range(HLOC):
                    hsl = slice(hh * DHEAD, (hh + 1) * DHEAD)
                    st_ps = st_ps_pool.tile([P, 512], F32, tag="stps")
                    nc.tensor.matmul(
                        st_ps[:], lhsT=qkT[hsl, 1, bi, ts(kt, P)],
                        rhs=qkT[hsl, 0, bi, ds(qb * 512, 512)],
                        start=True, stop=True,
                        tile_position=(hh * DHEAD, 0))
                    e_t = e_pool.tile([P, 512], BF16, tag="et")
                    nc.scalar.activation(e_t[:, c0:512], st_ps[:, c0:512],
                                         Act.Exp,
                                         scale=rk8_all[:, bi, kt,
                                                       hh:hh + 1])
                    if d >= 0:
                        nc.vector.tensor_tensor(
                            e_t[:, c0:c0 + P], e_t[:, c0:c0 + P], tri[:],
                            AluOp.mult)
                    nc.tensor.matmul(
                        o_ps[hh][:, c0:512],
                        lhsT=v_sb[:, bi, kt, hh, :],
                        rhs=e_t[:, c0:512],
                        start=(kt == 0), stop=(kt == nkt - 1))
            for hh in range(HLOC):
                nc.vector.tensor_copy(oU[:, qb % 2, hh, :], o_ps[hh][:])

        # ------- stage C: normalize half-batch, A2A, (out-proj later) ----
        def norm_half(bi, h, oU):
            # pack the 4 denominator rows at partitions 0 (hh=0) and 64
            # (hh=1) -> one full-rate reciprocal, then K=1 PE broadcast
            for q2 in range(2):
                for hh in range(HLOC):
                    nc.scalar.dma_start(
                        den[DHEAD * hh:DHEAD * hh + 1, q2, :],
                        oU[DHEAD:DHEAD + 1, q2, hh, :])
            nc.vector.reciprocal_approx_fast(den[:], den[:])
            with nc.allow_low_precision(
                    reason="bf16 softmax denom, rel-err budget"):
                nc.vector.tensor_copy(rden[:], den[:])
            for q2 in range(2):
                qb = 2 * h + q2
                for hh in range(HLOC):
                    bc_ps = bc_ps_pool.tile([DHEAD, 512], F32, tag="bc")
                    nc.tensor.matmul(
                        bc_ps[:],
                        lhsT=ones1[DHEAD * hh:DHEAD * hh + 1, :],
                        rhs=rden[DHEAD * hh:DHEAD * hh + 1, q2, :],
                        start=True, stop=True)
                    nc.vector.tensor_tensor(
                        oT_b[bi][hh * DHEAD:(hh + 1) * DHEAD,
                                 ds(qb * 512, 512)],
                        oU[0:DHEAD, q2, hh, :], bc_ps[:], AluOp.mult)
            nc.scalar.dma_start(
                cc_in[bi][h][:].rearrange("s p f -> p s f"),
                oT_b[bi][:, ds(h * 1024, 1024)]
                .rearrange("p (s f) -> p s f", f=GRAN))
            nc.gpsimd.collective_compute(
                "AllToAll", AluOp.bypass,
                replica_groups=[list(range(NCORES))],
                ins=[cc_in[bi][h].opt()], outs=[cc_out[bi][h].opt()])

        def outproj_half(bi, h):
            oA = oA_pool.tile([P, INNER // P, GRAN], BF16, tag="oA")
            nc.scalar.dma_start(
                oA[:], cc_out[bi][h][:].rearrange("s p f -> p s f"))
            yt = out_pool.tile([P, DIM], BF16, tag="yt")
            for half in range(2):
                out_ps = mm_ps.tile([P, 512], F32, tag="mm")
                for o in range(INNER // P):
                    nc.tensor.matmul(
                        out_ps[:], lhsT=oA[:, o, :],
                        rhs=w_out_sb[:, o, ds(half * 512, 512)],
                        start=(o == 0), stop=(o == INNER // P - 1))
                nc.vector.tensor_copy(yt[:, ds(half * 512, 512)], out_ps[:])
            nc.scalar.dma_start(y_out.ap()[bi, h], yt[:])

        # ---------------- the schedule ----------------
        for ti in range(KT_PER_B):              # QKV batch 0 + its x^2
            qkv_tile(0, ti)
            x2_square(0, ti)
        qkv_bend_qk(0)
        qkv_bend_v(0)
        for ti in range(KT_PER_B):              # QKV batch 1 + its x^2
            qkv_tile(1, ti)
            x2_square(1, ti)
        qkv_bend_qk(1)
        qkv_bend_v(1)
        for wo in range(2):     # deferred w_out load (needed at ~2/3 mark)
            nc.scalar.dma_start(w_out_sb[:, ds(wo * 4, 4), :],
                              w_out.ap()[:, ds(wo * 4, 4), :])

        oU0 = oU_pool.tile([1 + DHEAD, 2, HLOC, 512], F32, tag="oU")
        oT_b[0] = oT_pool.tile([P, N], BF16, tag="oTb", name="oT0")
        attn_qblock(0, 0, oU0)
        attn_qblock(0, 1, oU0)
        norm_half(0, 0, oU0)
        oU0b = oU_pool.tile([1 + DHEAD, 2, HLOC, 512], F32, tag="oU")
        attn_qblock(0, 2, oU0b)
        attn_qblock(0, 3, oU0b)
        norm_half(0, 1, oU0b)

        # batch 1: upper half first so the last A2A overlaps q0/q1 work
        oU1b = oU_pool.tile([1 + DHEAD, 2, HLOC, 512], F32, tag="oU")
        oT_b[1] = oT_pool.tile([P, N], BF16, tag="oTb", name="oT1")
        attn_qblock(1, 2, oU1b)
        attn_qblock(1, 3, oU1b)
        norm_half(1, 1, oU1b)
        outproj_half(0, 0)
        oU1 = oU_pool.tile([1 + DHEAD, 2, HLOC, 512], F32, tag="oU")
        attn_qblock(1, 0, oU1)
        attn_qblock(1, 1, oU1)
        norm_half(1, 0, oU1)
        outproj_half(0, 1)
        outproj_half(1, 1)
        outproj_half(1, 0)


# ----------------------------------------------------------------------
# Host side
# ----------------------------------------------------------------------

def make_in_maps(x, ln_w, ln_b, W_qkv, W_out):
    """Build the per-core input maps (host-side sharding/marshaling)."""
    import ml_dtypes
    x = np.asarray(x, dtype=np.float32)
    ln_w = np.asarray(ln_w, dtype=np.float32)
    ln_b = np.asarray(ln_b, dtype=np.float32)
    W_qkv = np.asarray(W_qkv, dtype=np.float32)
    W_out = np.asarray(W_out, dtype=np.float32)

    assert np.allclose(ln_b, 0.0), \
        "kernel folds ln_b@W into a bias; nonzero ln_b not wired up"

    x2d = np.ascontiguousarray(x.reshape(NTOK, DIM))
    x_rows = x2d.astype(ml_dtypes.float8_e4m3)
    # x^T stripes: [128 p, 8 o, 4096 t] with d = 128*o + p
    x_tr = np.ascontiguousarray(
        x2d.T.reshape(DIM // P, P, NTOK).transpose(1, 0, 2)
    ).astype(ml_dtypes.bfloat16)

    w_eff = ln_w[:, None] * W_qkv  # [1024, 3072]
    q_w = w_eff[:, 0 * INNER:1 * INNER]
    k_w = w_eff[:, 1 * INNER:2 * INNER]
    v_w = w_eff[:, 2 * INNER:3 * INNER]
    w_out_r = np.ascontiguousarray(
        W_out.reshape(INNER // P, P, DIM).transpose(1, 0, 2)
    ).astype(ml_dtypes.bfloat16)

    in_maps = []
    for c in range(NCORES):
        h0, h1 = 2 * c, 2 * c + 2
        wq = q_w[:, h0 * DHEAD:h1 * DHEAD]
        wk = k_w[:, h0 * DHEAD:h1 * DHEAD]
        wv = v_w[:, h0 * DHEAD:h1 * DHEAD]
        w_c = np.concatenate([wq, wk, wv], axis=1)      # [1024, 384]
        w_c = w_c - w_c.mean(axis=0, keepdims=True)     # fold LN mean-sub
        mu_col = np.full((DIM, 1), 1.0 / DIM, dtype=np.float32)
        w_c = np.concatenate([w_c, mu_col], axis=1)     # [1024, 385]
        w_c = np.ascontiguousarray(
            w_c.reshape(DIM // P, P, QKV_COLS + 1).transpose(1, 0, 2)
        ).astype(ml_dtypes.bfloat16)
        in_maps.append({
            "x_rows": x_rows,
            "x_tr": x_tr,
            "w_qkv": w_c,
            "w_out": w_out_r,
        })
    return in_maps


def gather_output(results):
    """results: list of per-core {name: array} -> full [2, 2048, 1024]."""
    full = np.empty((B, N, DIM), dtype=np.float32)
    for c in range(NCORES):
        part = np.asarray(results[c]["y_out"], dtype=np.float32)
        for bi in range(B):
            for h in range(2):
                t0 = h * 1024 + c * GRAN
                full[bi, t0:t0 + GRAN, :] = part[bi, h]
    return full


_NC_CACHE = None


def kernel(x, ln_w, ln_b, W_qkv, W_out):
    global _NC_CACHE
    from concourse.bass_utils import run_bass_kernel_spmd
    if _NC_CACHE is None:
        _NC_CACHE = build_kernel()
    in_maps = make_in_maps(x, ln_w, ln_b, W_qkv, W_out)
    res = run_bass_kernel_spmd(_NC_CACHE, in_maps,
                               core_ids=list(range(NCORES)))
    return gather_output(res.results)


# revision 37
# speedup vs baseline: 1.2053x; 1.2053x over previous
"""Trainium2 Bass kernel for causal cosine-sim attention block (8 cores), v6.

Reference computation:
  x [2, 2048, 1024] fp32
  xn = LayerNorm(x) * ln_w + ln_b
  qkv = xn @ W_qkv -> q, k, v   (16 heads x 64)
  q, k l2-normalized per head-dim; sim = (q.k) * 8, causal mask, softmax
  o = attn @ v ; out = o @ W_out   [2, 2048, 1024] fp32

Sharding (8 cores):
  - QKV projection + attention: head-parallel. Core c owns heads {2c, 2c+1}
    (column-sharded W_qkv).
  - Out projection: token-parallel over strided 128-token granules:
    granule g (tokens [128g, 128g+128)) of each batch-half belongs to core
    g % 8. Four small AllToAlls (one per batch-half) exchange head-sharded
    o^T for token granules, each overlapping subsequent attention compute.

v6 structure notes:
  - LN mean-subtraction is folded into W on the host (column-centered W);
    no on-chip rank-1 correction at all. A 385th W column of 1/1024 makes
    the QKV psum's last column the per-token mean (needed for variance).
  - rstd (for the V scale; it cancels for q,k under l2-norm) comes from
    sum(x^2) computed on DVE from x rows; var = ssq/1024 - mu^2.
  - x is fed twice from HBM: token-row tiles (for x^2) and transposed
    stripes x^T (matmul lhsT) -- no on-chip x transposes.
  - q is scaled by 1/||q|| on DVE; k stays raw and 8/||k|| rides the Exp
    activation's per-partition scale operand.
  - Softmax denominators (from the [V|1] PV matmul) are normalized via
    reciprocal + K=1 ones-matmul broadcast + DVE multiply.
  - PE warm-up: dummy matmuls at kernel start trip the HAM clock gate
    (cold 1.2 GHz -> warm 2.4 GHz) while input DMAs run.
  - Schedule: QKV(b0) -> [attention(b0) || QKV(b1)] -> [attention(b1) ||
    A2A(b0)+outproj(b0)] -> per-half A2A(b1)/outproj(b1) pipeline.
"""

import numpy as np

import concourse.bass as bass
import concourse.mybir as mybir
import concourse.tile as tile
from concourse import bacc
from concourse.bass import ts, ds

F32 = mybir.dt.float32
BF16 = mybir.dt.bfloat16
FP8 = mybir.dt.float8e4

NCORES = 8
DIM = 1024
HEADS = 16
DHEAD = 64
INNER = HEADS * DHEAD          # 1024
B = 2
N = 2048
NTOK = B * N                   # 4096
HLOC = HEADS // NCORES         # 2 heads per core
QKV_COLS = 3 * HLOC * DHEAD    # 384
EPS = 1e-5
SCALE = 8.0
P = 128
KT_PER_B = N // P              # 16 token tiles per batch
QB_PER_B = N // 512            # 4 q-blocks (512) per batch
GRAN = 128                     # out-proj token granule
AluOp = mybir.AluOpType
Act = mybir.ActivationFunctionType
AxX = mybir.AxisListType.X


def build_kernel():
    nc = bacc.Bacc("TRN2", target_bir_lowering=False, debug=False,
                   num_devices=NCORES)

    x_rows = nc.dram_tensor("x_rows", [NTOK, DIM], FP8,
                        kind="ExternalInput")
    x_tr = nc.dram_tensor("x_tr", [P, DIM // P, NTOK], BF16,
                          kind="ExternalInput")
    w_qkv = nc.dram_tensor("w_qkv", [P, DIM // P, QKV_COLS + 1], BF16,
                           kind="ExternalInput")
    w_out = nc.dram_tensor("w_out", [P, INNER // P, DIM], BF16,
                           kind="ExternalInput")
    y_out = nc.dram_tensor("y_out", [B, 2, GRAN, DIM], BF16,
                           kind="ExternalOutput")

    with tile.TileContext(nc) as tc:
        _body(nc, tc, x_rows, x_tr, w_qkv, w_out, y_out)
    nc.compile()
    return nc


def _body(nc, tc, x_rows, x_tr, w_qkv, w_out, y_out):
    import contextlib
    ctx = contextlib.ExitStack()
    with ctx:
        persist = ctx.enter_context(tc.tile_pool(name="persist", bufs=1))
        xt_pool = ctx.enter_context(tc.tile_pool(name="xt", bufs=2))
        xr_pool = ctx.enter_context(tc.tile_pool(name="xr", bufs=4))
        xr1_pool = ctx.enter_context(tc.tile_pool(name="xr1", bufs=2))
        sq_pool = ctx.enter_context(tc.tile_pool(name="sqd", bufs=2))
        qk_pool = ctx.enter_context(tc.tile_pool(name="qks", bufs=2))
        small = ctx.enter_context(tc.tile_pool(name="small", bufs=4))
        mm_ps = ctx.enter_context(
            tc.tile_pool(name="mmps", bufs=2, space="PSUM"))
        st_ps_pool = ctx.enter_context(
            tc.tile_pool(name="stps", bufs=2, space="PSUM"))
        o_ps_pool = ctx.enter_context(
            tc.tile_pool(name="ops", bufs=1, space="PSUM"))
        e_pool = ctx.enter_context(tc.tile_pool(name="epool", bufs=4))
        oU_pool = ctx.enter_context(tc.tile_pool(name="oU", bufs=2))
        oT_pool = ctx.enter_context(tc.tile_pool(name="oT", bufs=2))
        oA_pool = ctx.enter_context(tc.tile_pool(name="oA", bufs=2))
        out_pool = ctx.enter_context(tc.tile_pool(name="outp", bufs=2))
        norm_pool = ctx.enter_context(tc.tile_pool(name="norm", bufs=2))
        dram = ctx.enter_context(tc.tile_pool(name="dram", bufs=1,
                                              space="DRAM"))

        # ---------------- persistent SBUF ----------------
        w_qkv_sb = persist.tile([P, DIM // P, QKV_COLS + 1], BF16)
        w_out_sb = persist.tile([P, INNER // P, DIM], BF16)
        qkT = persist.tile([P, 2, B, N], BF16)                    # 16 KB
        v_sb = persist.tile([P, B, KT_PER_B, HLOC, DHEAD + 1], BF16)
        rk8_all = persist.tile([P, B, KT_PER_B, HLOC], F32)
        ones1 = persist.tile([DHEAD + 1, DHEAD], BF16)
        warm_a = persist.tile([P, P], BF16)
        warm_b = persist.tile([P, 512], BF16)
        tri = persist.tile([P, P], BF16)
        den = persist.tile([DHEAD + 1, 2, 512], F32)
        rden = persist.tile([DHEAD + 1, 2, 512], BF16)

        # upfront DMAs (Sync queue) + constants; w_out is deferred (only
        # needed ~2/3 into the kernel) so it doesn't block x loads.
        nc.scalar.dma_start(w_qkv_sb[:], w_qkv.ap())
        nc.vector.memset(ones1[:], 1.0)
        nc.vector.memset(warm_a[:], 0.0)
        nc.vector.memset(warm_b[:], 0.0)
        nc.vector.memset(den[:], 1.0)
        nc.vector.memset(v_sb[:, :, :, :, DHEAD], 1.0)
        nc.vector.memset(tri[:], 1.0)
        nc.gpsimd.affine_select(
            out=tri[:], in_=tri[:], pattern=[[1, P]],
            compare_op=AluOp.is_ge, fill=0.0, base=0,
            channel_multiplier=-1)

        # x^T in per-512-token-group tiles: QKV tile ti waits only on its
        # own group's 1MB DMA. Order: xt(b0), xr(b0) on Sync; xr(b1) on
        # Scalar; xt(b1) on Sync behind xr(b0).
        xr_view = x_rows.ap().rearrange("(n p) d -> p n d", p=P)
        xtg = [[persist.tile([P, DIM // P, 512], BF16, name=f"xtg{bi}{g}")
                for g in range(4)] for bi in range(B)]
        xr0 = [None] * 4
        xr1 = [None] * 4

        def xr_load(bi, g):
            pool = xr_pool if bi == 0 else xr1_pool
            xr = pool.tile([P, 4, DIM], FP8, tag=f"xr{bi}",
                           name=f"xr{bi}{g}")
            nc.scalar.dma_start(xr[:],
                                xr_view[:, ds(bi * KT_PER_B + 4 * g, 4), :])
            (xr0 if bi == 0 else xr1)[g] = xr

        for g in range(4):
            nc.sync.dma_start(xtg[0][g][:],
                              x_tr.ap()[:, :, ds(g * 512, 512)])
        for g in range(4):
            nc.sync.dma_start(xtg[1][g][:],
                              x_tr.ap()[:, :, ds(N + g * 512, 512)])
        xr_load(1, 0)
        xr_load(1, 1)
        for g in range(4):
            xr_load(0, g)

        # PE warm-up: dummy matmuls while DMAs land (~9 us of PE busy)
        warm_ps = mm_ps.tile([P, 512], F32, tag="mm")
        for i in range(20):
            nc.tensor.matmul(warm_ps[:], lhsT=warm_a[:], rhs=warm_b[:],
                             start=True, stop=True)

        # DRAM staging for the 4 AllToAlls (one per batch-half)
        cc_in = [[None] * 2 for _ in range(B)]
        cc_out = [[None] * 2 for _ in range(B)]
        for bi in range(B):
            for h in range(2):
                cc_in[bi][h] = dram.tile([NCORES, P, GRAN], BF16,
                                         name=f"cci{bi}{h}")
                cc_out[bi][h] = dram.tile([NCORES, P, GRAN], BF16,
                                          name=f"cco{bi}{h}")

        oT_b = [None] * B      # per-batch o^T (normalized, bf16)
        st = {}                # per-batch QKV staging tiles
        ssqx_all = persist.tile([P, B, KT_PER_B], F32)

        # ---- x^2: batch 0 on GpSimd(square)+DVE(reduce), batch 1 on ACT
        # Square+accum. Splits ~40us of elementwise work across engines;
        # all of it runs before the first Exp (no ACT table thrash).
        def x2_square(bi, t):
            g, j = divmod(t, 4)
            if bi == 1 and t in (4, 8):
                xr_load(1, 2 + (t == 8))
            xr = (xr0 if bi == 0 else xr1)[g]
            if bi == 0:
                dump = sq_pool.tile([P, DIM], BF16, tag="sqd")
                nc.scalar.activation(dump[:], xr[:, j, :], Act.Square,
                                     accum_out=ssqx_all[:, bi, t:t + 1])
            else:
                sqd = sq_pool.tile([P, DIM], BF16, tag="sqd")
                nc.gpsimd.tensor_tensor(sqd[:], xr[:, j, :], xr[:, j, :],
                                        AluOp.mult)
                nc.vector.reduce_sum(ssqx_all[:, bi, t:t + 1], sqd[:],
                                     axis=AxX)

        # ---------------- stage A: QKV for one 128-token tile ------------
        # Per tile: matmuls + raw evacs + squared sums only (no ACT, no
        # reciprocal) -- the norm math is batched per batch to avoid ACT
        # table thrash and per-tile reciprocal overhead.
        def qkv_tile(bi, ti):
            if ti == 0:
                st[bi] = dict(
                    qk_bf=qk_pool.tile([P, KT_PER_B, 2 * P], BF16,
                                       tag="qkbf", name=f"qkbf{bi}"),
                    mu=small.tile([P, KT_PER_B], F32, tag="muall",
                                  name=f"mu{bi}"),
                    ssq=small.tile([P, KT_PER_B, 4], F32, tag="ssqall",
                                   name=f"ssq{bi}"),
                )
            s = st[bi]

            qkv_ps = mm_ps.tile([P, QKV_COLS + 1], F32, tag="mm")
            for o in range(DIM // P):
                nc.tensor.matmul(qkv_ps[:],
                                 lhsT=xtg[bi][ti // 4][:, o, ts(ti % 4, P)],
                                 rhs=w_qkv_sb[:, o, :],
                                 start=(o == 0), stop=(o == DIM // P - 1))

            # raw evacs: mu col + q|k bf16 on DVE, v bf16 on ACT
            nc.vector.tensor_copy(s["mu"][:, ti:ti + 1],
                                  qkv_ps[:, QKV_COLS:QKV_COLS + 1])
            nc.vector.tensor_copy(s["qk_bf"][:, ti, :], qkv_ps[:, 0:2 * P])
            nc.vector.tensor_copy(
                v_sb[:, bi, ti, :, 0:DHEAD],
                qkv_ps[:, 2 * P:2 * P + 2 * DHEAD]
                .rearrange("p (h d) -> p h d", d=DHEAD))

            # squared norms per 64-col group
            sq = sq_pool.tile([P, 2 * P], BF16, tag="sq")
            nc.vector.tensor_tensor(sq[:], s["qk_bf"][:, ti, :],
                                    s["qk_bf"][:, ti, :], AluOp.mult)
            nc.vector.reduce_sum(
                s["ssq"][:, ti, :],
                sq[:].rearrange("p (j d) -> p j d", d=DHEAD), axis=AxX)

        # critical half of batch-end: q-norm scales + transposes
        def qkv_bend_qk(bi):
            s = st[bi]
            rq = small.tile([P, KT_PER_B, 2], F32, tag="rq")
            nc.scalar.activation(rq[:], s["ssq"][:, :, 0:2], Act.Sqrt)
            nc.vector.tensor_scalar_max(rq[:], rq[:], 1e-12)
            nc.vector.reciprocal_approx_fast(rq[:], rq[:])
            rk = small.tile([P, KT_PER_B, 2], F32, tag="rk")
            nc.scalar.activation(rk[:], s["ssq"][:, :, 2:4], Act.Sqrt,
                                 scale=1.0 / (SCALE * SCALE))
            nc.vector.tensor_scalar_max(rk[:], rk[:], 1e-12 / SCALE)
            nc.vector.reciprocal_approx_fast(rk8_all[:, bi, :, :], rk[:])
            for ti in range(KT_PER_B):
                for hh in range(HLOC):
                    nc.vector.tensor_scalar_mul(
                        s["qk_bf"][:, ti, ts(hh, DHEAD)],
                        s["qk_bf"][:, ti, ts(hh, DHEAD)],
                        rq[:, ti, hh:hh + 1])
                eng = nc.sync if ti % 2 == 0 else nc.scalar
                eng.dma_start_transpose(qkT[:, :, bi, ts(ti, P)],
                                        s["qk_bf"][:, ti, :])

        # relaxed half: rstd chain + v scales (needed by first PV only)
        def qkv_bend_v(bi):
            s = st[bi]
            musq = small.tile([P, KT_PER_B], F32, tag="musq")
            nc.vector.tensor_tensor(musq[:], s["mu"][:], s["mu"][:],
                                    AluOp.mult)
            varr = small.tile([P, KT_PER_B], F32, tag="varr")
            nc.vector.tensor_scalar(varr[:], ssqx_all[:, bi, :], 1.0 / DIM,
                                    EPS, AluOp.mult, AluOp.add)
            nc.vector.tensor_tensor(varr[:], varr[:], musq[:],
                                    AluOp.subtract)
            rstd = small.tile([P, KT_PER_B], F32, tag="rstd")
            nc.scalar.activation(rstd[:], varr[:], Act.Sqrt)
            nc.vector.reciprocal_approx_fast(rstd[:], rstd[:])
            for ti in range(KT_PER_B):
                nc.vector.tensor_scalar_mul(
                    v_sb[:, bi, ti, :, 0:DHEAD],
                    v_sb[:, bi, ti, :, 0:DHEAD], rstd[:, ti:ti + 1])

        # ------- stage B: attention for one 512-token q block ------------
        def attn_qblock(bi, qb, oU):
            o_ps = []
            for hh in range(HLOC):
                o_ps.append(o_ps_pool.tile([1 + DHEAD, 512], F32,
                                           tag=f"ops{hh}", name=f"ops{hh}"))
            nkt = 4 * (qb + 1)
            for kt in range(nkt):
                d = kt - 4 * qb
                c0 = max(d, 0) * P
                for hh in range(HLOC):
                    hsl = slice(hh * DHEAD, (hh + 1) * DHEAD)
                    st_ps = st_ps_pool.tile([P, 512], F32, tag="stps")
                    nc.tensor.matmul(
                        st_ps[:], lhsT=qkT[hsl, 1, bi, ts(kt, P)],
                        rhs=qkT[hsl, 0, bi, ds(qb * 512, 512)],
                        start=True, stop=True,
                        tile_position=(hh * DHEAD, 0))
                    e_t = e_pool.tile([P, 512], BF16, tag="et")
                    nc.scalar.activation(e_t[:, c0:512], st_ps[:, c0:512],
                                         Act.Exp,
                                         scale=rk8_all[:, bi, kt,
                                                       hh:hh + 1])
                    if d >= 0:
                        nc.vector.tensor_tensor(
                            e_t[:, c0:c0 + P], e_t[:, c0:c0 + P], tri[:],
                            AluOp.mult)
                    nc.tensor.matmul(
                        o_ps[hh][:, c0:512],
                        lhsT=v_sb[:, bi, kt, hh, :],
                        rhs=e_t[:, c0:512],
                        start=(kt == 0), stop=(kt == nkt - 1))
            for hh in range(HLOC):
                nc.vector.tensor_copy(oU[:, qb % 2, hh, :], o_ps[hh][:])

        # ------- stage C: normalize half-batch, A2A, (out-proj later) ----
        def norm_half(bi, h, oU):
            # pack the 4 denominator rows at partitions 0 (hh=0) and 64
            # (hh=1) -> one full-rate reciprocal, then K=1 PE broadcast
            for q2 in range(2):
                for hh in range(HLOC):
                    nc.scalar.dma_start(
                        den[DHEAD * hh:DHEAD * hh + 1, q2, :],
                        oU[DHEAD:DHEAD + 1, q2, hh, :])
            nc.vector.reciprocal_approx_fast(den[:], den[:])
            with nc.allow_low_precision(
                    reason="bf16 softmax denom, rel-err budget"):
                nc.vector.tensor_copy(rden[:], den[:])
            for q2 in range(2):
                qb = 2 * h + q2
                for hh in range(HLOC):
                    bc_ps = st_ps_pool.tile([DHEAD, 512], F32, tag="stps")
                    nc.tensor.matmul(
                        bc_ps[:],
                        lhsT=ones1[DHEAD * hh:DHEAD * hh + 1, :],
                        rhs=rden[DHEAD * hh:DHEAD * hh + 1, q2, :],
                        start=True, stop=True)
                    nc.vector.tensor_tensor(
                        oT_b[bi][hh * DHEAD:(hh + 1) * DHEAD,
                                 ds(qb * 512, 512)],
                        oU[0:DHEAD, q2, hh, :], bc_ps[:], AluOp.mult)
            nc.scalar.dma_start(
                cc_in[bi][h][:].rearrange("s p f -> p s f"),
                oT_b[bi][:, ds(h * 1024, 1024)]
                .rearrange("p (s f) -> p s f", f=GRAN))
            nc.gpsimd.collective_compute(
                "AllToAll", AluOp.bypass,
                replica_groups=[list(range(NCORES))],
                ins=[cc_in[bi][h].opt()], outs=[cc_out[bi][h].opt()])

        def outproj_half(bi, h):
            oA = oA_pool.tile([P, INNER // P, GRAN], BF16, tag="oA")
            nc.sync.dma_start(
                oA[:], cc_out[bi][h][:].rearrange("s p f -> p s f"))
            yt = out_pool.tile([P, DIM], BF16, tag="yt")
            for half in range(2):
                out_ps = mm_ps.tile([P, 512], F32, tag="mm")
                for o in range(INNER // P):
                    nc.tensor.matmul(
                        out_ps[:], lhsT=oA[:, o, :],
                        rhs=w_out_sb[:, o, ds(half * 512, 512)],
                        start=(o == 0), stop=(o == INNER // P - 1))
                nc.vector.tensor_copy(yt[:, ds(half * 512, 512)], out_ps[:])
            nc.sync.dma_start(y_out.ap()[bi, h], yt[:])

        # ---------------- the schedule ----------------
        for ti in range(KT_PER_B):              # QKV batch 0 + its x^2
            qkv_tile(0, ti)
            x2_square(0, ti)
        qkv_bend_qk(0)
        qkv_bend_v(0)

        oU0 = oU_pool.tile([1 + DHEAD, 2, HLOC, 512], F32, tag="oU")
        oT_b[0] = oT_pool.tile([P, N], BF16, tag="oTb", name="oT0")
        attn_qblock(0, 0, oU0)
        attn_qblock(0, 1, oU0)
        norm_half(0, 0, oU0)

        for ti in range(KT_PER_B):              # QKV batch 1 + its x^2
            qkv_tile(1, ti)
            x2_square(1, ti)
        qkv_bend_qk(1)
        qkv_bend_v(1)
        for wo in range(2):     # deferred w_out load
            nc.scalar.dma_start(w_out_sb[:, ds(wo * 4, 4), :],
                              w_out.ap()[:, ds(wo * 4, 4), :])

        oU0b = oU_pool.tile([1 + DHEAD, 2, HLOC, 512], F32, tag="oU")
        attn_qblock(0, 2, oU0b)
        attn_qblock(0, 3, oU0b)
        norm_half(0, 1, oU0b)

        # batch 1: upper half first so the last A2A overlaps trailing work
        oU1b = oU_pool.tile([1 + DHEAD, 2, HLOC, 512], F32, tag="oU")
        oT_b[1] = oT_pool.tile([P, N], BF16, tag="oTb", name="oT1")
        attn_qblock(1, 2, oU1b)
        attn_qblock(1, 3, oU1b)
        norm_half(1, 1, oU1b)
        oU1 = oU_pool.tile([1 + DHEAD, 2, HLOC, 512], F32, tag="oU")
        attn_qblock(1, 0, oU1)
        attn_qblock(1, 1, oU1)
        norm_half(1, 0, oU1)
        outproj_half(0, 0)
        outproj_half(0, 1)
        outproj_half(1, 1)
        outproj_half(1, 0)
